# revision 8
# baseline (speedup 1.0000x reference)
"""Masked phase-locking value (PLV) kernel for Trainium2, 8 NeuronCores.

Math: out[b] = |sum_ij M_ij * exp(i*(a_bi - b_bj))| / max(sum(M), 1)

Device decomposition (per core, Na sharded 8 ways -> 1024 i-columns each):
    Z[c, i] = sum_j cs2[j, c] * M[i, j]        (TensorE, fp8 DoubleRow)
with cs2 = [cb; sb] stacked along c (c = 2B = 128) as the STATIONARY
operand and the transposed mask streaming through. The whole Nb=8192
contraction accumulates in PSUM (one bank per i-block), so the epilogue
shrinks to the sharded dim:
    racc[c] = sum_i Z[c, i] * WA[c, i]          (DVE mult + ACT accumulate)
    qacc[c] = sum_i Z[c, i] * WS[c, i]
with WA = [ca; sa], WS = [sa; -ca] (sign folded in on host), giving
real_b = sum_cores racc[b] + racc[64+b], imag_b likewise from qacc.

dtypes: mask 0/1 and cs2 in fp8e4 -> DoubleRow double-pumped matmul
(2 contraction rows/cycle); WA/WS fp8; PSUM/epilogue fp32. Host-emulated
end-to-end rel err 4.0e-3 (gate 2e-2).

The kernel is HBM-DMA-bound (~9.3 MB/core at ~330 GB/s): mask tiles are
one-shot (whole shard fits SBUF - no reuse hazards, DMAs all queue at
t=0 and stream back-to-back); i-blocks are sized [448, 448, 128] so the
final accumulation chain and epilogue are short; groups within an
i-block are small-big so the PE starts early; a PE warm-up burst during
the DMA lead-in defeats the HAM cold-clock penalty.
"""

import numpy as np

import concourse.bass as bass
import concourse.tile as tile
from concourse import bacc, mybir
from concourse.bass_utils import run_bass_kernel_spmd

B = 64
NA = 8192
NB = 8192
NCORES = 8
NASH = NA // NCORES          # i-columns per core
JC = NB // 256               # 32 DoubleRow j-chunks of 256

IBS = [448, 448, 128]        # i-block widths (each <= 512-wide PSUM bank)
IBOFF = [0, 448, 896]
assert sum(IBS) == NASH
# j-chunks per mask DMA group, per i-block: small first (early PE start),
# tiny last (short tail before the final epilogue)
GJC = [[4, 12, 16], [8, 16, 8], [28, 4]]
assert all(sum(g) == JC for g in GJC)

NWU = 6                      # PE warm-up matmuls during DMA lead-in

F8 = mybir.dt.float8e4
F32 = mybir.dt.float32
DR = mybir.MatmulPerfMode.DoubleRow
COPY_F = mybir.ActivationFunctionType.Copy


def build_program() -> bass.Bass:
    nc = bacc.Bacc("TRN2")
    # mask: concatenated group blocks, each contiguous [128, gjc, 2, ibw]
    mask_d = nc.dram_tensor("mask", [128 * JC * 2 * NASH], F8, kind="ExternalInput")
    # cs2: 2 contiguous pieces of [128, {4,28}, 2, 128]
    cs2_d = nc.dram_tensor("cs2", [128 * JC * 2 * 128], F8, kind="ExternalInput")
    wv_d = nc.dram_tensor("wv", [128, 2, NASH], F8, kind="ExternalInput")
    out_d = nc.dram_tensor("out", [128, 2 * len(IBS)], F32, kind="ExternalOutput")

    with tile.TileContext(nc) as tc:
        with (
            tc.tile_pool(name="consts", bufs=1) as consts,
            tc.tile_pool(name="masks", bufs=1) as masks,
            tc.tile_pool(name="scratch", bufs=2) as scratch,
            tc.tile_pool(name="junk", bufs=2) as junkp,
            tc.tile_pool(name="zpsum", bufs=1, space="PSUM") as zpool,
            tc.tile_pool(name="wups", bufs=1, space="PSUM") as wu_pool,
        ):
            # scalar HWDGE ring: cs2 head piece, cs2 tail, wv
            cs2_sb = consts.tile([128, JC, 2, 128], F8)
            for p0, pjc in ((0, 4), (4, 28)):
                src = cs2_d[p0 * 256 * 128 : (p0 + pjc) * 256 * 128].rearrange(
                    "(p j t c) -> p j t c", p=128, j=pjc, t=2
                )
                nc.scalar.dma_start(out=cs2_sb[:, p0 : p0 + pjc], in_=src)
            wv_sb = consts.tile([128, 2, NASH], F8)
            nc.scalar.dma_start(out=wv_sb[:], in_=wv_d[:])

            # sync HWDGE ring: all mask groups, one-shot tiles, queued at t=0
            mts = []
            off = 0
            for ib, ibw in enumerate(IBS):
                for g, gjc in enumerate(GJC[ib]):
                    sz = 128 * gjc * 2 * ibw
                    mt = masks.tile([128, gjc, 2, ibw], F8, tag=f"mask{ib}_{g}")
                    src = mask_d[off : off + sz].rearrange(
                        "(p k t i) -> p k t i", p=128, k=gjc, t=2
                    )
                    nc.sync.dma_start(out=mt[:], in_=src)
                    mts.append(mt)
                    off += sz

            # PE warm-up on a memset tile while the first mask group is in
            # flight (HAM clock ramp)
            wu8 = consts.tile([128, 128], F8)
            nc.vector.memset(wu8[:], 1.0)
            wuR = consts.tile([128, 2, 512], F8)
            nc.vector.memset(wuR[:], 1.0)
            wu_ps = wu_pool.tile([128, 512], F32)
            for r in range(NWU):
                nc.tensor.matmul(
                    out=wu_ps[:], lhsT=wu8[:], rhs=wuR[:, 0],
                    start=(r == 0), stop=(r == NWU - 1),
                )

            racc = consts.tile([128, 2 * len(IBS)], F32)
            ti = 0
            for ib, ibw in enumerate(IBS):
                z = zpool.tile([128, ibw], F32, tag=f"z{ib}")
                jc = 0
                for gjc in GJC[ib]:
                    mt = mts[ti]
                    ti += 1
                    for k in range(gjc):
                        nc.tensor.matmul(
                            out=z[:],
                            lhsT=cs2_sb[:, jc],
                            rhs=mt[:, k],
                            start=(jc == 0),
                            stop=(jc == JC - 1),
                            perf_mode=DR,
                        )
                        jc += 1
                isl = slice(IBOFF[ib], IBOFF[ib] + ibw)
                for q in range(2):  # 0: real-side (WA), 1: imag-side (WS)
                    pr = scratch.tile([128, ibw], F32, tag="prod")
                    nc.vector.tensor_mul(out=pr[:], in0=z[:], in1=wv_sb[:, q, isl])
                    jr = junkp.tile([128, ibw], F32, tag="junk")
                    nc.scalar.activation(
                        out=jr[:], in_=pr[:], func=COPY_F,
                        accum_out=racc[:, 2 * ib + q : 2 * ib + q + 1],
                    )
            nc.scalar.dma_start(out=out_d[:], in_=racc[:])
    nc.finalize()
    return nc


def prep_inputs(phases_a, phases_b, coupling_mask):
    pa = np.asarray(phases_a, dtype=np.float32)
    pb = np.asarray(phases_b, dtype=np.float32)
    ca, sa = np.cos(pa), np.sin(pa)
    cb, sb = np.cos(pb), np.sin(pb)
    f8np = mybir.dt.np(F8)

    # cs2 pieces: [p, jc, t, c] with j = jc*256 + t*128 + p, piece-contiguous
    CS2 = np.concatenate([cb, sb], axis=0).astype(f8np)     # [c, j]
    csv = CS2.T.reshape(JC, 2, 128, 128)                    # [jc, t, p, c]
    cs2_host = np.concatenate(
        [
            np.ascontiguousarray(csv[a:b].transpose(2, 0, 1, 3)).reshape(-1)
            for a, b in ((0, 4), (4, JC))
        ]
    )

    one_byte = np.array([1.0], f8np).view(np.uint8)[0]
    mask_u8 = (np.asarray(coupling_mask) != 0).astype(np.uint8) * one_byte
    MT = np.ascontiguousarray(mask_u8.T)                    # [j, i]

    WA = np.concatenate([ca, sa], axis=0)                   # [c, i_full]
    WS = np.concatenate([sa, -ca], axis=0)

    in_maps = []
    for c in range(NCORES):
        isl = slice(c * NASH, (c + 1) * NASH)
        v = MT[:, isl].reshape(JC, 2, 128, NASH)            # [jc, t, p, i]
        blocks = []
        for ib, ibw in enumerate(IBS):
            sub = v[:, :, :, IBOFF[ib] : IBOFF[ib] + ibw]
            jc = 0
            for gjc in GJC[ib]:
                blk = sub[jc : jc + gjc].transpose(2, 0, 1, 3)  # [p, k, t, ii]
                blocks.append(np.ascontiguousarray(blk).reshape(-1))
                jc += gjc
        m_host = np.concatenate(blocks).view(f8np)
        wv = np.stack([WA[:, isl], WS[:, isl]], axis=1).astype(f8np)
        in_maps.append({"mask": m_host, "cs2": cs2_host, "wv": wv})
    return in_maps


def combine(outs, coupling_mask):
    o = np.stack(outs).astype(np.float64)   # [NCORES, 128, 2*len(IBS)]
    r = o[:, :, 0::2].sum(axis=2)           # [NCORES, 128]
    q = o[:, :, 1::2].sum(axis=2)
    real = (r[:, :B] + r[:, B:]).sum(axis=0)
    imag = (q[:, :B] + q[:, B:]).sum(axis=0)
    n_pairs = max(float(np.asarray(coupling_mask).sum()), 1.0)
    return (np.sqrt(real * real + imag * imag) / n_pairs).astype(np.float32)


_prog_cache: list = []


def kernel(phases_a, phases_b, coupling_mask):
    in_maps = prep_inputs(phases_a, phases_b, coupling_mask)
    if not _prog_cache:
        _prog_cache.append(build_program())
    res = run_bass_kernel_spmd(_prog_cache[0], in_maps, core_ids=list(range(NCORES)))
    return combine([r["out"] for r in res.results], coupling_mask)
